# revision 9
# baseline (speedup 1.0000x reference)
"""Cluster-wise linear (MoE-style dense routing) Trainium2 kernel.

Computes out[t,o] = sum_c prob[t,c] * (x[t] @ W[c].T + b[c])[o] for
x (128,321,336) f32, prob (128,321,8), W (8,96,336), b (8,96).

Strategy: data-parallel over 8 NeuronCores (tokens = batch*n_vars split
evenly). Per core, 128-token tiles:
  - gpsimd DMA loads x with inline f32->bf16 cast
  - DMA xbar transposes put the contraction dim on partitions
  - 6 bf16 matmuls accumulate Y[t, o*8+c] = (x|1) @ Wt_aug (bias folded
    in via a ones column; weights packed o-major on host)
  - ScalarE evicts Y PSUM->SBUF bf16
  - VectorE: Z = Y * prob (stride-0 broadcast), then strided reduce over
    the cluster axis -> out[t, o] in f32
"""

import numpy as np
import ml_dtypes

import concourse.bass as bass
import concourse.mybir as mybir
import concourse.tile as tile
from concourse.bass_utils import run_bass_kernel_spmd

N_CORES = 8
BSZ, N_VARS, IN_DIM, OUT_DIM, N_CLUSTER = 128, 321, 336, 96, 8
TOK = BSZ * N_VARS            # 41088
TPC = TOK // N_CORES          # 5136 tokens per core
P = 128
N_TILES = (TPC + P - 1) // P  # 41 (40 full + 1 tail of 16)
TAIL = TPC - (N_TILES - 1) * P  # 16
IN_P = 384                    # padded input dim: 336 data + 1 ones + 47 zeros
CO = OUT_DIM * N_CLUSTER      # 768, o-major: co = o*8 + c


def split_multi_waits(nc):
    """This walrus build only supports one sync-wait per instruction; hoist
    extra waits onto same-engine nops inserted immediately before."""
    n_split = 0
    for fn in nc.m.functions:
        for bb in fn.blocks:
            insts = bb.instructions
            out = []
            changed = False
            for inst in insts:
                si = inst.sync_info
                if si is not None and si.on_wait and len(si.on_wait) > 1:
                    waits = list(si.on_wait)
                    del si.on_wait[1:]
                    si.on_wait[0] = waits[-1]
                    for w in waits[:-1]:
                        nop = mybir.InstNoOp(
                            name=f"{inst.name}-wsplit-{n_split}", ins=[], outs=[]
                        )
                        n_split += 1
                        nop.engine = inst.engine
                        nop.sync_info = mybir.SyncInfo(on_wait=[w], on_update=[])
                        out.append(nop)
                        changed = True
                out.append(inst)
            if changed:
                insts[:] = out
    return n_split


def build_nc(nrep: int = 1, bufs: int = 3, n_tiles: int = N_TILES, tail: int = TAIL, split_waits: bool = True):
    tpc = (n_tiles - 1) * P + tail
    nc = bass.Bass()
    x_d = nc.dram_tensor("x", [tpc, IN_DIM], mybir.dt.float32, kind="ExternalInput")
    p_d = nc.dram_tensor(
        "probp", [P, n_tiles * N_CLUSTER], mybir.dt.bfloat16, kind="ExternalInput"
    )
    w_d = nc.dram_tensor("wt", [IN_P, CO], mybir.dt.bfloat16, kind="ExternalInput")
    o_d = nc.dram_tensor("out", [tpc, OUT_DIM], mybir.dt.float32, kind="ExternalOutput")

    dt = mybir.dt
    with tile.TileContext(nc) as tc:
        with (
            tc.tile_pool(name="const", bufs=1) as const,
            tc.tile_pool(name="work", bufs=1) as work,
            tc.tile_pool(name="psum", bufs=1, space="PSUM") as psum,
        ):
            # one-time loads
            wtb = const.tile([P, 3 * CO], dt.bfloat16)
            wtb3 = wtb.rearrange("p (k n) -> p k n", k=3)
            nc.gpsimd.dma_start(wtb3[:], w_d.rearrange("(k p) n -> p k n", p=P))
            pball = const.tile([P, n_tiles * N_CLUSTER], dt.bfloat16)
            nc.gpsimd.dma_start(pball[:], p_d[:])
            pb3 = pball.rearrange("p (j c) -> p j c", c=N_CLUSTER)

            # rings
            xb_ring = [work.tile([P, IN_P], dt.bfloat16, name=f"xb{i}") for i in range(bufs)]
            xT_ring = [
                work.tile([P, P], dt.bfloat16, name=f"xT{i}") for i in range(3 * bufs)
            ]
            y_ring = [
                psum.tile([P, CO], dt.float32, name=f"yps{i}") for i in range(bufs)
            ]
            ysb_ring = [
                work.tile([P, CO], dt.bfloat16, name=f"ysb{i}") for i in range(bufs)
            ]
            z_ring = [work.tile([P, CO], dt.bfloat16, name=f"z{i}") for i in range(bufs)]
            o_ring = [
                work.tile([P, OUT_DIM], dt.float32, name=f"osb{i}") for i in range(bufs)
            ]
            # preset the ones column (bias row after transpose) and zero pad
            for xb in xb_ring:
                nc.vector.memset(xb[:, IN_DIM : IN_DIM + 1], 1.0)
                nc.vector.memset(xb[:, IN_DIM + 1 :], 0.0)

            def tile_body(j: int):
                h = P if j < n_tiles - 1 else tail
                t0 = j * P
                xb = xb_ring[j % bufs]
                nc.gpsimd.dma_start(xb[:h, 0:IN_DIM], x_d[t0 : t0 + h, :])
                xTs = [xT_ring[(3 * j + k) % (3 * bufs)] for k in range(3)]
                for k in range(3):
                    nc.sync.dma_start(
                        xTs[k][:, 0:h], xb[0:h, k * P : (k + 1) * P], transpose=True
                    )
                yps = y_ring[j % bufs]
                for k in range(3):
                    for n0, n1 in ((0, 512), (512, CO)):
                        nc.tensor.matmul(
                            yps[:h, n0:n1],
                            xTs[k][:, 0:h],
                            wtb3[:, k, n0:n1],
                            start=(k == 0),
                            stop=(k == 2),
                        )
                ysb = ysb_ring[j % bufs]
                nc.scalar.copy(ysb[:h, :], yps[:h, :])
                z = z_ring[j % bufs]
                zv = z[0:h].rearrange("p (o c) -> p o c", c=N_CLUSTER)
                yv = ysb[0:h].rearrange("p (o c) -> p o c", c=N_CLUSTER)
                pbc = pb3[0:h, j, :].unsqueeze(1).broadcast_to([h, OUT_DIM, N_CLUSTER])
                nc.vector.tensor_tensor(zv, yv, pbc, mybir.AluOpType.mult)
                osb = o_ring[j % bufs]
                nc.vector.tensor_reduce(
                    osb[0:h], zv, mybir.AxisListType.X, mybir.AluOpType.add
                )
                nc.gpsimd.dma_start(o_d[t0 : t0 + h, :], osb[0:h])

            def sweep(_iv=None):
                for j in range(n_tiles):
                    tile_body(j)

            for _ in range(nrep):
                sweep()

    if split_waits:
        split_multi_waits(nc)
    return nc


def pack_inputs(x, prob, W, b):
    """Host-side packing. Returns per-core input maps."""
    x = np.asarray(x, dtype=np.float32).reshape(TOK, IN_DIM)
    prob = np.asarray(prob, dtype=np.float32).reshape(TOK, N_CLUSTER)
    W = np.asarray(W, dtype=np.float32)
    b = np.asarray(b, dtype=np.float32)

    # weights: wt[i, o*8+c] = W[c,o,i]; bias row at i=336; zeros to IN_P
    wt = np.zeros((IN_P, CO), dtype=np.float32)
    wt[:IN_DIM] = W.transpose(2, 1, 0).reshape(IN_DIM, CO)
    wt[IN_DIM] = b.T.reshape(CO)
    wt16 = np.ascontiguousarray(wt.astype(ml_dtypes.bfloat16))

    in_maps = []
    for c in range(N_CORES):
        xs = np.ascontiguousarray(x[c * TPC : (c + 1) * TPC])
        ps = prob[c * TPC : (c + 1) * TPC]
        pp = np.zeros((N_TILES * P, N_CLUSTER), dtype=np.float32)
        pp[:TPC] = ps
        # (j, p, c) -> (p, j, c)
        pp = pp.reshape(N_TILES, P, N_CLUSTER).transpose(1, 0, 2)
        pp16 = np.ascontiguousarray(
            pp.astype(ml_dtypes.bfloat16).reshape(P, N_TILES * N_CLUSTER)
        )
        in_maps.append({"x": xs, "probp": pp16, "wt": wt16})
    return in_maps


_cached = {}


def kernel(x, prob, W, b):
    key = "main"
    if key not in _cached:
        _cached[key] = build_nc(nrep=1)
    nc = _cached[key]
    in_maps = pack_inputs(x, prob, W, b)
    res = run_bass_kernel_spmd(nc, in_maps, list(range(N_CORES)))
    outs = [res.results[c]["out"] for c in range(N_CORES)]
    out = np.concatenate(outs, axis=0).reshape(BSZ, N_VARS, OUT_DIM)
    return out.astype(np.float32)


if __name__ == "__main__":
    rng = np.random.default_rng(0)
    x = rng.standard_normal((BSZ, N_VARS, IN_DIM)).astype(np.float32)
    prob = rng.random((BSZ, N_VARS, N_CLUSTER)).astype(np.float32)
    W = (rng.standard_normal((N_CLUSTER, OUT_DIM, IN_DIM)) / 18.3).astype(np.float32)
    b = rng.standard_normal((N_CLUSTER, OUT_DIM)).astype(np.float32) / 18.3
    out = kernel(x, prob, W, b)
    ref = np.einsum("ti,coi,tc->to", x.reshape(TOK, IN_DIM), W,
                    prob.reshape(TOK, N_CLUSTER)) + prob.reshape(TOK, N_CLUSTER) @ b
    ref = ref.reshape(BSZ, N_VARS, OUT_DIM)
    err = np.linalg.norm(out - ref) / np.linalg.norm(ref)
    print("rel_l2:", err)


# revision 13
# speedup vs baseline: 5.8438x; 5.8438x over previous
"""Cluster-wise linear (MoE-style dense routing) Trainium2 kernel.

Computes out[t,o] = sum_c prob[t,c] * (x[t] @ W[c].T + b[c])[o] for
x (128,321,336) f32, prob (128,321,8), W (8,96,336), b (8,96).

Strategy: data-parallel over 8 NeuronCores (tokens = batch*n_vars split
evenly). Per core, 128-token tiles:
  - gpsimd DMA loads x with inline f32->bf16 cast
  - DMA xbar transposes put the contraction dim on partitions
  - 6 bf16 matmuls accumulate Y[t, o*8+c] = (x|1) @ Wt_aug (bias folded
    in via a ones column; weights packed o-major on host)
  - ScalarE evicts Y PSUM->SBUF bf16
  - VectorE: Z = Y * prob (stride-0 broadcast), then strided reduce over
    the cluster axis -> out[t, o] in f32
"""

import numpy as np
import ml_dtypes

import concourse.bass as bass
import concourse.mybir as mybir
import concourse.tile as tile
from concourse.bass_utils import run_bass_kernel_spmd
from concourse.masks import make_identity

N_CORES = 8
BSZ, N_VARS, IN_DIM, OUT_DIM, N_CLUSTER = 128, 321, 336, 96, 8
TOK = BSZ * N_VARS            # 41088
TPC = TOK // N_CORES          # 5136 tokens per core
P = 128
N_TILES = (TPC + P - 1) // P  # 41 (40 full + 1 tail of 16)
TAIL = TPC - (N_TILES - 1) * P  # 16
IN_P = 384                    # padded input dim: 336 data + 1 ones + 47 zeros
CO = OUT_DIM * N_CLUSTER      # 768, o-major: co = o*8 + c


def split_multi_waits(nc):
    """This walrus build only supports one sync-wait per instruction; hoist
    extra waits onto same-engine nops inserted immediately before."""
    n_split = 0
    for fn in nc.m.functions:
        for bb in fn.blocks:
            insts = bb.instructions
            out = []
            changed = False
            for inst in insts:
                si = inst.sync_info
                if si is not None and si.on_wait and len(si.on_wait) > 1:
                    waits = list(si.on_wait)
                    del si.on_wait[1:]
                    si.on_wait[0] = waits[-1]
                    for w in waits[:-1]:
                        nop = mybir.InstNoOp(
                            name=f"{inst.name}-wsplit-{n_split}", ins=[], outs=[]
                        )
                        n_split += 1
                        nop.engine = inst.engine
                        nop.sync_info = mybir.SyncInfo(on_wait=[w], on_update=[])
                        out.append(nop)
                        changed = True
                out.append(inst)
            if changed:
                insts[:] = out
    return n_split


def build_nc(nrep: int = 1, bufs: int = 3, n_tiles: int = N_TILES, tail: int = TAIL, split_waits: bool = True,
             do_load=True, do_transpose=True, do_matmul=True, do_stage2=True):
    tpc = (n_tiles - 1) * P + tail
    nc = bass.Bass()
    x_d = nc.dram_tensor("x", [tpc, IN_DIM], mybir.dt.float32, kind="ExternalInput")
    p_d = nc.dram_tensor(
        "probp", [P, n_tiles * N_CLUSTER], mybir.dt.bfloat16, kind="ExternalInput"
    )
    w_d = nc.dram_tensor("wt", [IN_P, CO], mybir.dt.bfloat16, kind="ExternalInput")
    o_d = nc.dram_tensor("out", [tpc, OUT_DIM], mybir.dt.float32, kind="ExternalOutput")

    dt = mybir.dt
    with tile.TileContext(nc) as tc:
        with (
            tc.tile_pool(name="const", bufs=1) as const,
            tc.tile_pool(name="work", bufs=1) as work,
            tc.tile_pool(name="psum", bufs=1, space="PSUM") as psum,
        ):
            # one-time loads
            wtb = const.tile([P, 3 * CO], dt.bfloat16)
            wtb3 = wtb.rearrange("p (k n) -> p k n", k=3)
            nc.gpsimd.dma_start(wtb3[:], w_d.rearrange("(k p) n -> p k n", p=P))
            pball = const.tile([P, n_tiles * N_CLUSTER], dt.bfloat16)
            nc.gpsimd.dma_start(pball[:], p_d[:])
            pb3 = pball.rearrange("p (j c) -> p j c", c=N_CLUSTER)
            ident = const.tile([P, P], dt.bfloat16)
            make_identity(nc, ident[:])

            # rings
            xb_ring = [work.tile([P, IN_P], dt.bfloat16, name=f"xb{i}") for i in range(bufs)]
            xT_ring = [
                work.tile([P, 3 * P], dt.bfloat16, name=f"xT{i}") for i in range(bufs)
            ]
            tps_ring = [
                psum.tile([P, 3 * P], dt.bfloat16, name=f"tps{i}") for i in range(2)
            ]
            y_ring = [
                psum.tile([P, CO], dt.float32, name=f"yps{i}") for i in range(bufs)
            ]
            ysb_ring = [
                work.tile([P, CO], dt.bfloat16, name=f"ysb{i}") for i in range(bufs)
            ]
            z_ring = [work.tile([P, CO], dt.bfloat16, name=f"z{i}") for i in range(bufs)]
            o_ring = [
                work.tile([P, OUT_DIM], dt.float32, name=f"osb{i}") for i in range(bufs)
            ]
            # preset the ones column (bias row after transpose) and zero pad
            for xb in xb_ring:
                if do_load:
                    nc.vector.memset(xb[:, IN_DIM : IN_DIM + 1], 1.0)
                    nc.vector.memset(xb[:, IN_DIM + 1 :], 0.0)
                else:
                    nc.vector.memset(xb[:], 0.0)
            if not do_transpose:
                for t in xT_ring:
                    nc.vector.memset(t[:], 0.0)
            if not do_stage2:
                for t in o_ring:
                    nc.vector.memset(t[:], 0.0)

            def tile_body(j: int):
                h = P if j < n_tiles - 1 else tail
                t0 = j * P
                xb = xb_ring[j % bufs]
                if do_load:
                    nc.gpsimd.dma_start(xb[:h, 0:IN_DIM], x_d[t0 : t0 + h, :])
                xT = xT_ring[j % bufs]
                if do_transpose:
                    tps = tps_ring[j % 2]
                    for k in range(3):
                        nc.tensor.transpose(
                            tps[:, k * P : k * P + h],
                            xb[0:h, k * P : (k + 1) * P],
                            ident[0:h, 0:h],
                        )
                    nc.vector.tensor_copy(xT[:], tps[:])
                yps = y_ring[j % bufs]
                if do_matmul:
                    for k in range(3):
                        for n0, n1 in ((0, 512), (512, CO)):
                            nc.tensor.matmul(
                                yps[:h, n0:n1],
                                xT[:, k * P : k * P + h],
                                wtb3[:, k, n0:n1],
                                start=(k == 0),
                                stop=(k == 2),
                            )
                ysb = ysb_ring[j % bufs]
                osb = o_ring[j % bufs]
                if do_stage2:
                    nc.scalar.copy(ysb[:h, :], yps[:h, :])
                    z = z_ring[j % bufs]
                    zv = z[0:h].rearrange("p (o c) -> p o c", c=N_CLUSTER)
                    yv = ysb[0:h].rearrange("p (o c) -> p o c", c=N_CLUSTER)
                    pbc = pb3[0:h, j, :].unsqueeze(1).broadcast_to([h, OUT_DIM, N_CLUSTER])
                    nc.vector.tensor_tensor(zv, yv, pbc, mybir.AluOpType.mult)
                    nc.vector.tensor_reduce(
                        osb[0:h], zv, mybir.AxisListType.X, mybir.AluOpType.add
                    )
                nc.gpsimd.dma_start(o_d[t0 : t0 + h, :], osb[0:h])

            def sweep(_iv=None):
                for j in range(n_tiles):
                    tile_body(j)

            for _ in range(nrep):
                sweep()

    if split_waits:
        split_multi_waits(nc)
    return nc


def pack_inputs(x, prob, W, b):
    """Host-side packing. Returns per-core input maps."""
    x = np.asarray(x, dtype=np.float32).reshape(TOK, IN_DIM)
    prob = np.asarray(prob, dtype=np.float32).reshape(TOK, N_CLUSTER)
    W = np.asarray(W, dtype=np.float32)
    b = np.asarray(b, dtype=np.float32)

    # weights: wt[i, o*8+c] = W[c,o,i]; bias row at i=336; zeros to IN_P
    wt = np.zeros((IN_P, CO), dtype=np.float32)
    wt[:IN_DIM] = W.transpose(2, 1, 0).reshape(IN_DIM, CO)
    wt[IN_DIM] = b.T.reshape(CO)
    wt16 = np.ascontiguousarray(wt.astype(ml_dtypes.bfloat16))

    in_maps = []
    for c in range(N_CORES):
        xs = np.ascontiguousarray(x[c * TPC : (c + 1) * TPC])
        ps = prob[c * TPC : (c + 1) * TPC]
        pp = np.zeros((N_TILES * P, N_CLUSTER), dtype=np.float32)
        pp[:TPC] = ps
        # (j, p, c) -> (p, j, c)
        pp = pp.reshape(N_TILES, P, N_CLUSTER).transpose(1, 0, 2)
        pp16 = np.ascontiguousarray(
            pp.astype(ml_dtypes.bfloat16).reshape(P, N_TILES * N_CLUSTER)
        )
        in_maps.append({"x": xs, "probp": pp16, "wt": wt16})
    return in_maps


_cached = {}


def kernel(x, prob, W, b):
    key = "main"
    if key not in _cached:
        _cached[key] = build_nc(nrep=1)
    nc = _cached[key]
    in_maps = pack_inputs(x, prob, W, b)
    res = run_bass_kernel_spmd(nc, in_maps, list(range(N_CORES)))
    outs = [res.results[c]["out"] for c in range(N_CORES)]
    out = np.concatenate(outs, axis=0).reshape(BSZ, N_VARS, OUT_DIM)
    return out.astype(np.float32)


if __name__ == "__main__":
    rng = np.random.default_rng(0)
    x = rng.standard_normal((BSZ, N_VARS, IN_DIM)).astype(np.float32)
    prob = rng.random((BSZ, N_VARS, N_CLUSTER)).astype(np.float32)
    W = (rng.standard_normal((N_CLUSTER, OUT_DIM, IN_DIM)) / 18.3).astype(np.float32)
    b = rng.standard_normal((N_CLUSTER, OUT_DIM)).astype(np.float32) / 18.3
    out = kernel(x, prob, W, b)
    ref = np.einsum("ti,coi,tc->to", x.reshape(TOK, IN_DIM), W,
                    prob.reshape(TOK, N_CLUSTER)) + prob.reshape(TOK, N_CLUSTER) @ b
    ref = ref.reshape(BSZ, N_VARS, OUT_DIM)
    err = np.linalg.norm(out - ref) / np.linalg.norm(ref)
    print("rel_l2:", err)
